# revision 1
# baseline (speedup 1.0000x reference)
"""Trainium2 Bass kernel for nn_AttentionNet (GNN message passing, 2-layer GCN
with edge-MLP attention weights), 8 NeuronCores, tgt-sharded.

Strategy:
  - Core k owns target nodes [k*12500, (k+1)*12500) and ALL their in-edges.
  - Host: per-core node permutation (lexsort by per-chunk in-degree) ->
    node-major slot structure: per (tile t of 128 nodes, src-chunk c) a
    rectangular block of Dbar[t,c] slot-columns; real edges fill lanes,
    pads get a valid dummy index (0) and mask 0.
  - Device per core:
    stage A: edge MLP (ew = sigmoid(relu(exT@W1+b1)@W2+b2)) in wrap layout.
    deg = per-tile reduce of masked ew; dinv = 1/sqrt(deg+1).
    xs = dinv * (x @ Wc1) -> AllGather -> table1 [100352, 128] bf16.
    L1: per chunk, dma_gather rows, mult by ew, strided segment reduce -> acc1.
    h1s = dinv*relu(dinv*(acc1 + xs) + bc1) -> AllGather -> table2.
    L2: same gathers on table2 -> acc2.
    out = log_softmax(dinv*((acc2 + h1s) @ Wc2) + bc2).
"""
import sys
import numpy as np

sys.path.insert(0, "/opt/trn_rl_repo")

import ml_dtypes
import concourse.bass as bass
import concourse.tile as tile
import concourse.bacc as bacc
from concourse import mybir
from concourse.bass_utils import run_bass_kernel_spmd

NC = 8
N = 100000
NB = 12500
NBP = 12544           # padded per-core nodes (98 * 128)
NT = NBP // 128       # 98 node tiles
CH = 25088            # chunk size in global padded table (2 cores * 12544)
TBL = NC * NBP        # 100352
P = 128
EF, EFIL = 16, 32
NF, NFIL, CLS = 128, 64, 16
CALL_COLS = 64        # gather call width (<= 64 cols = 8192 idxs)

F32 = mybir.dt.float32
BF16 = mybir.dt.bfloat16
I16 = mybir.dt.int16
AF = mybir.ActivationFunctionType
OP = mybir.AluOpType


def _prep(x, edge_index, edge_x, W1, b1, W2, b2, Wc1, bc1, Wc2, bc2):
    src = np.asarray(edge_index[0]).astype(np.int64)
    tgt = np.asarray(edge_index[1]).astype(np.int64)
    x = np.asarray(x, np.float32)
    edge_x = np.asarray(edge_x, np.float32)

    core_of_tgt = tgt // NB
    chunk_of_src_node = np.minimum(src // (2 * NB), 3)  # chunk c = cores 2c,2c+1

    # --- per-core node perms (lexsort by per-chunk in-degree desc) ---
    perms, poss, degcs = [], [], []
    core_edges = []
    for k in range(NC):
        m = core_of_tgt == k
        e_ids = np.nonzero(m)[0]
        t_loc = tgt[e_ids] - k * NB
        ch = chunk_of_src_node[e_ids]
        d = np.zeros((NBP, 4), np.int64)
        np.add.at(d, (t_loc, ch), 1)
        perm = np.lexsort((-d[:, 3], -d[:, 2], -d[:, 1], -d[:, 0]))
        pos = np.empty(NBP, np.int64)
        pos[perm] = np.arange(NBP)
        perms.append(perm); poss.append(pos); degcs.append(d)
        core_edges.append((e_ids, t_loc, ch))

    # global table row for any src node
    src_core = src // NB
    # pos within its own core
    pos_all = np.empty(N, np.int64)
    for k in range(NC):
        ids = np.arange(k * NB, (k + 1) * NB)
        pos_all[ids] = poss[k][ids - k * NB]
    grow = src_core * NBP + pos_all[src]          # global row of src
    idx16_of_edge = grow - chunk_of_src_node * CH  # < 25088

    # --- per-(tile, chunk) Dbar, common across cores ---
    Dbar = np.zeros((NT, 4), np.int64)
    for k in range(NC):
        d_sorted = degcs[k][perms[k]].reshape(NT, P, 4)
        Dbar = np.maximum(Dbar, d_sorted.max(1))
    Wc_cols = Dbar.sum(0)                  # per-chunk total columns
    base_c = np.zeros(5, np.int64)
    base_c[1:] = np.cumsum(Wc_cols)
    SC_raw = int(base_c[4])
    SC = ((SC_raw + 11) // 12) * 12        # pad to x12 for stage-A batching
    EP = SC * P
    # per (c, t) column offset
    coloff = np.zeros((4, NT), np.int64)
    for c in range(4):
        coloff[c] = base_c[c] + np.concatenate(([0], np.cumsum(Dbar[:, c])[:-1]))

    # --- per-core slot arrays ---
    in_maps = []
    for k in range(NC):
        e_ids, t_loc, ch = core_edges[k]
        pos_t = poss[k][t_loc]             # slot lane/tile of target
        tt = pos_t // P
        pp = pos_t % P
        # rank within (node, chunk): sort by (ch, pos_t) then cumcount
        order = np.lexsort((pos_t, ch))
        ch_s, pos_s = ch[order], pos_t[order]
        key = ch_s * NBP + pos_s
        newgrp = np.concatenate(([True], key[1:] != key[:-1]))
        grp_start = np.maximum.accumulate(np.where(newgrp, np.arange(len(key)), 0))
        kk = np.arange(len(key)) - grp_start
        rank = np.empty(len(key), np.int64)
        rank[order] = kk
        col = coloff[ch, tt] + rank
        slot = col * P + pp

        exT = np.zeros((17, EP), ml_dtypes.bfloat16)
        exT[:16, slot] = edge_x[e_ids].T.astype(ml_dtypes.bfloat16)
        exT[16, slot] = 1.0
        unw = np.zeros(EP, np.int16)
        unw[slot] = idx16_of_edge[e_ids].astype(np.int16)
        w16 = unw.reshape(EP // 16, 16).T
        idxw = np.tile(w16, (8, 1)).copy()
        mask = np.zeros((P, SC), ml_dtypes.bfloat16)
        mask[pp, col] = 1.0
        xt = np.zeros((P, NBP), ml_dtypes.bfloat16)
        xrows = x[k * NB:(k + 1) * NB]     # [12500, 128]
        # node at pos i is perm[i]; cols are pos-ordered
        pvals = perms[k]
        valid = pvals < NB
        xt[:, valid.nonzero()[0]] = xrows[pvals[valid]].T.astype(ml_dtypes.bfloat16)
        in_maps.append({
            "exT": np.asarray(exT), "idxw": np.asarray(idxw),
            "maskw": np.asarray(mask), "xT": np.asarray(xt),
        })

    consts = dict(
        W1a=np.vstack([np.asarray(W1, np.float32),
                       np.asarray(b1, np.float32)[None, :]]).astype(ml_dtypes.bfloat16),
        W2r4=np.tile(np.asarray(W2, np.float32), (4, 1)).astype(ml_dtypes.bfloat16),
        Wc1=np.asarray(Wc1, np.float32).astype(ml_dtypes.bfloat16),
        Wc2=np.asarray(Wc2, np.float32).astype(ml_dtypes.bfloat16),
        bc1r=np.tile(np.asarray(bc1, np.float32)[None, :], (P, 1)),
        bc2r=np.tile(np.asarray(bc2, np.float32)[None, :], (P, 1)),
        ident=np.eye(P, dtype=ml_dtypes.bfloat16),
        b2f=float(np.asarray(b2, np.float32).reshape(-1)[0]),
    )
    meta = dict(Dbar=Dbar, coloff=coloff, base_c=base_c, SC=SC, EP=EP,
                perms=perms, poss=poss)
    return in_maps, consts, meta


def _build(consts, meta):
    Dbar, coloff, SC, EP = meta["Dbar"], meta["coloff"], meta["SC"], meta["EP"]
    nc = bacc.Bacc("TRN2", target_bir_lowering=False, debug=False, num_devices=NC)

    exT_d = nc.dram_tensor("exT", [17, EP], BF16, kind="ExternalInput")
    idxw_d = nc.dram_tensor("idxw", [P, EP // 16], I16, kind="ExternalInput")
    mask_d = nc.dram_tensor("maskw", [P, SC], BF16, kind="ExternalInput")
    xT_d = nc.dram_tensor("xT", [P, NBP], BF16, kind="ExternalInput")
    out_d = nc.dram_tensor("out", [NBP, CLS], F32, kind="ExternalOutput")

    W1a_d = nc.inline_tensor(consts["W1a"], "W1a")
    W2r4_d = nc.inline_tensor(consts["W2r4"], "W2r4")
    Wc1_d = nc.inline_tensor(consts["Wc1"], "Wc1")
    Wc2_d = nc.inline_tensor(consts["Wc2"], "Wc2")
    bc1r_d = nc.inline_tensor(consts["bc1r"], "bc1r")
    bc2r_d = nc.inline_tensor(consts["bc2r"], "bc2r")
    ident_d = nc.inline_tensor(np.asarray(consts["ident"]), "ident")
    b2f = consts["b2f"]

    with tile.TileContext(nc) as tc:
        with (
            tc.tile_pool(name="persist", bufs=1) as pers,
            tc.tile_pool(name="stream", bufs=2) as strm,
            tc.tile_pool(name="ps", bufs=2, space="PSUM") as psp,
            tc.tile_pool(name="ps1", bufs=1, space="PSUM") as psp1,
            tc.tile_pool(name="dram", bufs=1, space="DRAM") as drp,
        ):
            # ---- persistent tiles ----
            ew = pers.tile([P, SC], BF16)
            maskt = pers.tile([P, SC], BF16)
            xTt = pers.tile([P, NBP], BF16)
            acc1 = pers.tile([P, NT * NFIL], F32)
            acc2 = pers.tile([P, NT * NFIL], F32)
            xs_loc = pers.tile([P, NT * NFIL], BF16)
            h1s_loc = pers.tile([P, NT * NFIL], BF16)
            deg4 = pers.tile([P, 4 * NT], F32)
            dinv = pers.tile([P, NT], F32)
            scr = pers.tile([P, NT], F32)
            W1s = pers.tile([17, EFIL], BF16)
            W2s = pers.tile([P, 1], BF16)
            Wc1s = pers.tile([P, NFIL], BF16)
            Wc2s = pers.tile([NFIL, CLS], BF16)
            bc1s = pers.tile([P, NFIL], F32)
            bc2s = pers.tile([P, CLS], F32)
            idents = pers.tile([P, P], BF16)
            zeros = pers.tile([P, NT * NFIL], BF16)

            nc.sync.dma_start(maskt[:], mask_d[:])
            nc.sync.dma_start(xTt[:], xT_d[:])
            nc.sync.dma_start(W1s[:], W1a_d[:])
            nc.sync.dma_start(W2s[:], W2r4_d[:])
            nc.sync.dma_start(Wc1s[:], Wc1_d[:])
            nc.sync.dma_start(Wc2s[:], Wc2_d[:])
            nc.sync.dma_start(bc1s[:], bc1r_d[:])
            nc.sync.dma_start(bc2s[:], bc2r_d[:])
            nc.sync.dma_start(idents[:], ident_d[:])
            nc.vector.memset(zeros[:], 0)
            nc.vector.memset(acc1[:], 0)
            nc.vector.memset(acc2[:], 0)

            # ---- DRAM: bounce + tables ----
            bounce1 = drp.tile([TBL // NC, P], BF16)
            table1 = drp.tile([TBL, P], BF16)
            bounce2 = drp.tile([TBL // NC, P], BF16)
            table2 = drp.tile([TBL, P], BF16)

            # ---- stage A: edge MLP (3 groups of 512 per batch; psum base 0/32/64) ----
            nbatch = EP // 1536
            for b in range(nbatch):
                ext = strm.tile([17, 1536], BF16)
                nc.sync.dma_start(ext[:], exT_d[:, b * 1536:(b + 1) * 1536])
                h4p = psp.tile([96, 512], F32, space="PSUM")
                for g in range(3):
                    nc.tensor.matmul(
                        out=h4p[32 * g:32 * (g + 1), :],
                        lhsT=W1s[:], rhs=ext[:, 512 * g:512 * (g + 1)],
                        start=True, stop=True)
                h4s = strm.tile([96, 512], BF16)
                nc.scalar.activation(out=h4s[:], in_=h4p[:], func=AF.Relu)
                ewp = psp.tile([P, 12], F32, space="PSUM")
                for cl in range(12):
                    g, q = cl // 4, cl % 4
                    nc.tensor.matmul(
                        out=ewp[:, cl:cl + 1],
                        lhsT=h4s[32 * g:32 * (g + 1), 128 * q:128 * (q + 1)],
                        rhs=W2s[32 * g:32 * (g + 1), :],
                        start=True, stop=True)
                nc.scalar.activation(out=ew[:, b * 12:(b + 1) * 12], in_=ewp[:],
                                     func=AF.Sigmoid, bias=b2f)
            # mask pads
            nc.vector.tensor_tensor(out=ew[:], in0=ew[:], in1=maskt[:], op=OP.mult)

            # ---- deg / dinv ----
            for c in range(4):
                for t in range(NT):
                    D = int(Dbar[t, c])
                    if D == 0:
                        nc.vector.memset(deg4[:, c * NT + t:c * NT + t + 1], 0)
                        continue
                    o = int(coloff[c, t])
                    nc.vector.tensor_reduce(
                        out=deg4[:, c * NT + t:c * NT + t + 1],
                        in_=ew[:, o:o + D], axis=mybir.AxisListType.X, op=OP.add)
            nc.vector.tensor_reduce(
                out=dinv[:],
                in_=deg4[:].rearrange("p (c t) -> p c t", c=4).transpose([0, 2, 1]),
                axis=mybir.AxisListType.X, op=OP.add)
            # dinv = 1/sqrt(deg+1)
            nc.scalar.activation(out=scr[:], in_=dinv[:], func=AF.Sqrt, bias=1.0)
            nc.vector.reciprocal(out=dinv[:], in_=scr[:])

            # ---- xs = dinv * (x @ Wc1), write bounce1 ----
            for t in range(NT):
                xp = psp1.tile([P, NFIL], F32, space="PSUM")
                nc.tensor.matmul(out=xp[:], lhsT=xTt[:, t * P:(t + 1) * P],
                                 rhs=Wc1s[:], start=True, stop=True)
                nc.scalar.activation(out=xs_loc[:, t * NFIL:(t + 1) * NFIL],
                                     in_=xp[:], func=AF.Copy,
                                     scale=dinv[:, t:t + 1])
            nc.sync.dma_start(
                bounce1[:, :NFIL].rearrange("(t p) f -> p t f", p=P),
                xs_loc[:].rearrange("p (t f) -> p t f", f=NFIL))
            # zero the pad cols 64:128 once
            nc.sync.dma_start(
                bounce1[:, NFIL:].rearrange("(t p) f -> p t f", p=P),
                zeros[:].rearrange("p (t f) -> p t f", f=NFIL))
            nc.gpsimd.collective_compute(
                "AllGather", OP.bypass, replica_groups=[list(range(NC))],
                ins=[bounce1[:].opt()], outs=[table1[:].opt()])

            # ---- gather+reduce loop (shared for L1/L2) ----
            def layer_loop(table, acc):
                for c in range(4):
                    a = int(coloff[c, 0])
                    end_c = int(coloff[c, NT - 1] + Dbar[NT - 1, c])
                    o = a
                    while o < end_c:
                        w = min(CALL_COLS, end_c - o)
                        ni = w * P
                        idxt = strm.tile([P, w * 8], I16)
                        nc.sync.dma_start(idxt[:], idxw_d[:, o * 8:(o + w) * 8])
                        msgs = strm.tile([P, w, P], BF16)
                        nc.gpsimd.dma_gather(
                            out_ap=msgs[:], in_ap=table[c * CH:(c + 1) * CH, :],
                            idxs_ap=idxt[:], num_idxs=ni, num_idxs_reg=ni,
                            elem_size=P, single_packet=False)
                        scl = strm.tile([P, w, NFIL], BF16)
                        nc.vector.tensor_tensor(
                            out=scl[:],
                            in0=msgs[:, :, :NFIL],
                            in1=ew[:, o:o + w].unsqueeze(2).to_broadcast(
                                [P, w, NFIL]),
                            op=OP.mult)
                        # per-tile blocks inside [o, o+w)
                        for t in range(NT):
                            bs = int(coloff[c, t]); be = bs + int(Dbar[t, c])
                            lo, hi = max(bs, o), min(be, o + w)
                            if lo >= hi:
                                continue
                            D = hi - lo
                            tmp = strm.tile([P, NFIL], F32)
                            nc.vector.tensor_reduce(
                                out=tmp[:],
                                in_=scl[:, lo - o:hi - o, :].transpose([0, 2, 1]),
                                axis=mybir.AxisListType.X, op=OP.add)
                            nc.vector.tensor_tensor(
                                out=acc[:, t * NFIL:(t + 1) * NFIL],
                                in0=acc[:, t * NFIL:(t + 1) * NFIL],
                                in1=tmp[:], op=OP.add)
                        o += w

            layer_loop(table1, acc1)

            # ---- h1s ----
            for t in range(NT):
                sl = slice(t * NFIL, (t + 1) * NFIL)
                t1 = strm.tile([P, NFIL], F32)
                nc.vector.tensor_tensor(out=t1[:], in0=acc1[:, sl],
                                        in1=xs_loc[:, sl], op=OP.add)
                t2 = strm.tile([P, NFIL], F32)
                nc.scalar.activation(out=t2[:], in_=t1[:], func=AF.Copy,
                                     scale=dinv[:, t:t + 1])
                nc.vector.tensor_tensor(out=t2[:], in0=t2[:], in1=bc1s[:],
                                        op=OP.add)
                nc.vector.tensor_scalar_max(t2[:], t2[:], 0.0)
                nc.scalar.activation(out=h1s_loc[:, sl], in_=t2[:], func=AF.Copy,
                                     scale=dinv[:, t:t + 1])
            nc.sync.dma_start(
                bounce2[:, :NFIL].rearrange("(t p) f -> p t f", p=P),
                h1s_loc[:].rearrange("p (t f) -> p t f", f=NFIL))
            nc.sync.dma_start(
                bounce2[:, NFIL:].rearrange("(t p) f -> p t f", p=P),
                zeros[:].rearrange("p (t f) -> p t f", f=NFIL))
            nc.gpsimd.collective_compute(
                "AllGather", OP.bypass, replica_groups=[list(range(NC))],
                ins=[bounce2[:].opt()], outs=[table2[:].opt()])

            layer_loop(table2, acc2)

            # ---- final: out = log_softmax(dinv*((acc2+h1s)@Wc2)+bc2) ----
            for t in range(NT):
                sl = slice(t * NFIL, (t + 1) * NFIL)
                u = strm.tile([P, NFIL], BF16)
                nc.vector.tensor_tensor(out=u[:], in0=acc2[:, sl],
                                        in1=h1s_loc[:, sl], op=OP.add)
                utp = psp1.tile([NFIL, P], BF16, space="PSUM")
                nc.tensor.transpose(out=utp[:], in_=u[:], identity=idents[:])
                uts = strm.tile([NFIL, P], BF16)
                nc.vector.tensor_copy(out=uts[:], in_=utp[:])
                vp = psp1.tile([P, CLS], F32, space="PSUM")
                nc.tensor.matmul(out=vp[:], lhsT=uts[:], rhs=Wc2s[:],
                                 start=True, stop=True)
                z = strm.tile([P, CLS], F32)
                nc.scalar.activation(out=z[:], in_=vp[:], func=AF.Copy,
                                     scale=dinv[:, t:t + 1])
                nc.vector.tensor_tensor(out=z[:], in0=z[:], in1=bc2s[:],
                                        op=OP.add)
                nmx = strm.tile([P, 1], F32)
                nc.vector.tensor_reduce(out=nmx[:], in_=z[:],
                                        axis=mybir.AxisListType.X, op=OP.max,
                                        negate=True)
                et = strm.tile([P, CLS], F32)
                sume = strm.tile([P, 1], F32)
                nc.scalar.activation(out=et[:], in_=z[:], func=AF.Exp,
                                     bias=nmx[:], accum_out=sume[:])
                lse = strm.tile([P, 1], F32)
                nc.scalar.activation(out=lse[:], in_=sume[:], func=AF.Ln)
                res = strm.tile([P, CLS], F32)
                nc.vector.tensor_scalar(out=res[:], in0=z[:], scalar1=nmx[:],
                                        scalar2=lse[:], op0=OP.add,
                                        op1=OP.subtract)
                nc.sync.dma_start(out_d[t * P:(t + 1) * P, :], res[:])

    nc.compile()
    return nc


_last = {}


def kernel(**inputs):
    in_maps, consts, meta = _prep(**inputs)
    nc = _build(consts, meta)
    _last.update(nc=nc, in_maps=in_maps, meta=meta)
    res = run_bass_kernel_spmd(nc, in_maps, core_ids=list(range(NC)))
    _last["exec_time_ns"] = getattr(res, "exec_time_ns", None)
    out = np.zeros((N, CLS), np.float32)
    for k in range(NC):
        ok = res.results[k]["out"]          # [NBP, CLS] pos-ordered
        perm = meta["perms"][k]
        valid = perm < NB
        out[k * NB + perm[valid]] = ok[valid.nonzero()[0]]
    return out



# revision 5
# speedup vs baseline: 1.6285x; 1.6285x over previous
"""Trainium2 Bass kernel for nn_AttentionNet (GNN message passing, 2-layer GCN
with edge-MLP attention weights), 8 NeuronCores, tgt-sharded.

Strategy (v2 — upload-minimal, S-matrix scatter):
  - Core k owns target nodes [k*12500, (k+1)*12500) and ALL their in-edges.
  - Host packs this core's edges grouped by (src-chunk c, tgt-tile t) with
    per-(c,t) counts padded only to the cross-core max (few % inflation).
    Uploads per core: edge_x packed fp8 [16, ET], gather idx int16 (compact
    16-partition wrap), tgt%256 int16 wrap, x-slice fp8 [128, 12544].
  - Device per core:
    stage A: edge MLP ew = sigmoid(relu(ex@W1+b1)@W2+b2) over packed slots.
    deg pass: per 128-slot column build S[e,j] = (iota==tgt)*ew (one fused
    tensor_scalar), matmul S^T @ ones -> per-tile deg; dinv = rsqrt(deg+1).
    xs = dinv * (x @ Wc1) -> AllGather -> table1 [100352, 128] bf16.
    L1: windowed dma_gather of src rows; per column S^T @ msgs[:, :, :64]
    accumulated in PSUM per tile-run -> acc1.
    h1s = dinv*relu(dinv*(acc1+xs)+bc1); xw2 = h1s @ Wc2 -> table2.
    L2: same gathers on table2, rhs width 16 -> acc2.
    out = log_softmax(dinv*(acc2 + xw2_loc) + bc2) in fp16.
"""
import sys
import numpy as np

sys.path.insert(0, "/opt/trn_rl_repo")

import ml_dtypes
import concourse.bass as bass
import concourse.tile as tile
import concourse.bacc as bacc
from concourse import mybir
from concourse.bass_utils import run_bass_kernel_spmd

NC = 8
N = 100000
NB = 12500
NBP = 12544           # padded per-core nodes (98 * 128)
NT = NBP // 128       # 98 node tiles
CH = 25088            # chunk size in global padded table (2 cores * 12544)
TBL = NC * NBP        # 100352
P = 128
EF, EFIL = 16, 32
NF, NFIL, CLS = 128, 64, 16
CALL_COLS = 64        # gather call width (64 cols = 8192 idxs)
BATCH = 1536          # stage-A slots per batch (12 psum cols)

F32 = mybir.dt.float32
BF16 = mybir.dt.bfloat16
F16 = mybir.dt.float16
I16 = mybir.dt.int16
FP8 = mybir.dt.float8e4
AF = mybir.ActivationFunctionType
OP = mybir.AluOpType
FP8NP = mybir.dt.np(FP8)


def _prep(x, edge_index, edge_x, W1, b1, W2, b2, Wc1, bc1, Wc2, bc2):
    src = np.asarray(edge_index[0]).astype(np.int64)
    tgt = np.asarray(edge_index[1]).astype(np.int64)
    E = src.shape[0]
    x = np.asarray(x, np.float32)
    edge_x = np.asarray(edge_x, np.float32)

    core = tgt // NB
    local_t = tgt - core * NB                  # 0..12499
    tile_of = local_t >> 7                     # 0..97
    src_core = src // NB
    row = src_core * NBP + (src - src_core * NB)
    chunk = src_core >> 1                      # 0..3
    idxc = (row - chunk * CH).astype(np.int16)  # < 25088
    tgtm = (local_t & 255).astype(np.int16)

    cell = chunk * NT + tile_of                # 0..391
    keyk = core * (4 * NT) + cell
    cnt = np.bincount(keyk, minlength=NC * 4 * NT).reshape(NC, 4 * NT)
    M = cnt.max(0).reshape(4, NT)              # cross-core max per (chunk, tile)
    assert M.min() >= 128, "column may span >2 tiles; layout invalid"

    run_raw = M.sum(1)                         # slots per chunk run
    run_len = ((run_raw + 127) // 128) * 128
    chunk_base = np.zeros(5, np.int64)
    chunk_base[1:] = np.cumsum(run_len)
    off = np.zeros((4, NT), np.int64)
    off[:, 1:] = np.cumsum(M, 1)[:, :-1]
    ET_raw = int(chunk_base[4])
    ET = ((ET_raw + BATCH - 1) // BATCH) * BATCH
    SCOLS = ET // 128

    # rank of each edge within its (core, chunk, tile) group
    order = np.argsort(keyk, kind="stable")
    ks = keyk[order]
    newgrp = np.r_[True, ks[1:] != ks[:-1]]
    gstart = np.maximum.accumulate(np.where(newgrp, np.arange(E), 0))
    rank = np.empty(E, np.int64)
    rank[order] = np.arange(E) - gstart
    slot = chunk_base[chunk] + off[chunk, tile_of] + rank

    in_maps = []
    for k in range(NC):
        m = core == k
        sl = slot[m]
        ex2 = np.zeros((ET, 16), FP8NP)
        ex2[sl] = edge_x[m].astype(FP8NP)
        iw = np.zeros(ET, np.int16)
        iw[sl] = idxc[m]
        tw = np.full(ET, -1, np.int16)
        tw[sl] = tgtm[m]
        xt = np.zeros((P, NBP), FP8NP)
        xt[:, :NB] = x[k * NB:(k + 1) * NB].T.astype(FP8NP)
        in_maps.append({
            "exT": np.ascontiguousarray(ex2.T),
            "idx16": np.ascontiguousarray(iw.reshape(ET // 16, 16).T),
            "tgtm": np.ascontiguousarray(tw.reshape(SCOLS, 128).T),
            "xT": xt,
        })

    consts = dict(
        W1=np.asarray(W1, np.float32).astype(ml_dtypes.bfloat16),
        b1r3=np.tile(np.asarray(b1, np.float32)[:, None], (3, 1)),
        W2r4=np.tile(np.asarray(W2, np.float32), (4, 1)).astype(ml_dtypes.bfloat16),
        Wc1=np.asarray(Wc1, np.float32).astype(ml_dtypes.bfloat16),
        Wc2=np.asarray(Wc2, np.float32).astype(ml_dtypes.bfloat16),
        bc1r=np.tile(np.asarray(bc1, np.float32)[None, :], (P, 1)),
        bc2r=np.tile(np.asarray(bc2, np.float32)[None, :], (P, 1)),
        iota2d=np.tile(np.arange(256, dtype=np.float32)[None, :], (P, 1)),
        ident=np.eye(P, dtype=ml_dtypes.bfloat16),
        b2f=float(np.asarray(b2, np.float32).reshape(-1)[0]),
    )
    meta = dict(M=M, off=off, chunk_base=chunk_base, run_len=run_len,
                run_raw=run_raw, ET=ET, SCOLS=SCOLS)
    return in_maps, consts, meta


def _build(consts, meta):
    M, off = meta["M"], meta["off"]
    chunk_base, run_len = meta["chunk_base"], meta["run_len"]
    ET, SCOLS = meta["ET"], meta["SCOLS"]
    nc = bacc.Bacc("TRN2", target_bir_lowering=False, debug=False, num_devices=NC)

    exT_d = nc.dram_tensor("exT", [16, ET], FP8, kind="ExternalInput")
    idx16_d = nc.dram_tensor("idx16", [16, ET // 16], I16, kind="ExternalInput")
    tgtm_d = nc.dram_tensor("tgtm", [P, SCOLS], I16, kind="ExternalInput")
    xT_d = nc.dram_tensor("xT", [P, NBP], FP8, kind="ExternalInput")
    out_d = nc.dram_tensor("out", [NBP, CLS], F16, kind="ExternalOutput")

    W1_d = nc.inline_tensor(consts["W1"], "W1")
    b1r3_d = nc.inline_tensor(consts["b1r3"], "b1r3")
    W2r4_d = nc.inline_tensor(consts["W2r4"], "W2r4")
    Wc1_d = nc.inline_tensor(consts["Wc1"], "Wc1")
    Wc2_d = nc.inline_tensor(consts["Wc2"], "Wc2")
    bc1r_d = nc.inline_tensor(consts["bc1r"], "bc1r")
    bc2r_d = nc.inline_tensor(consts["bc2r"], "bc2r")
    iota_d = nc.inline_tensor(consts["iota2d"], "iota2d")
    ident_d = nc.inline_tensor(np.asarray(consts["ident"]), "ident")
    b2f = consts["b2f"]

    # per-chunk tile column ranges (inclusive), raw (unaligned) ends
    tile_cols = []
    for c in range(4):
        tc_c = []
        for t in range(NT):
            s0 = int(off[c, t])
            s1 = s0 + int(M[c, t])
            tc_c.append((t, s0 // 128, (s1 - 1) // 128))
        tile_cols.append(tc_c)

    with tile.TileContext(nc) as tc:
        with (
            tc.tile_pool(name="persist", bufs=1) as pers,
            tc.tile_pool(name="stream", bufs=2) as strm,
            tc.tile_pool(name="spool", bufs=4) as spl,
            tc.tile_pool(name="psA", bufs=2, space="PSUM") as psA,
            tc.tile_pool(name="psG", bufs=2, space="PSUM") as psG,
            tc.tile_pool(name="psT", bufs=2, space="PSUM") as psT,
            tc.tile_pool(name="dram", bufs=1, space="DRAM") as drp,
        ):
            # ---- persistent tiles ----
            ewf = pers.tile([P, SCOLS], F32)
            tgtf = pers.tile([P, SCOLS], F32)
            idxr = pers.tile([P, ET // 16], I16)
            xTt = pers.tile([P, NBP], BF16)
            xs_loc = pers.tile([P, NT * NFIL], BF16)
            h1s_loc = pers.tile([P, NT * NFIL], BF16)
            xw2_loc = pers.tile([P, NT * CLS], BF16)
            acc1 = pers.tile([P, NT * NFIL], F32)
            acc2 = pers.tile([P, NT * CLS], F32)
            deg = pers.tile([P, NT], F32)
            dinv = pers.tile([P, NT], F32)
            scr = pers.tile([P, NT], F32)
            W1s = pers.tile([16, EFIL], BF16)
            b1r3s = pers.tile([96, 1], F32)
            W2s = pers.tile([P, 1], BF16)
            Wc1s = pers.tile([P, NFIL], BF16)
            Wc2s = pers.tile([NFIL, CLS], BF16)
            bc1s = pers.tile([P, NFIL], F32)
            bc2s = pers.tile([P, CLS], F32)
            iotas = pers.tile([P, 256], F32)
            idents = pers.tile([P, P], BF16)
            ones = pers.tile([P, 1], BF16)

            nc.sync.dma_start(W1s[:], W1_d[:])
            nc.sync.dma_start(b1r3s[:], b1r3_d[:])
            nc.sync.dma_start(W2s[:], W2r4_d[:])
            nc.sync.dma_start(Wc1s[:], Wc1_d[:])
            nc.sync.dma_start(Wc2s[:], Wc2_d[:])
            nc.sync.dma_start(bc1s[:], bc1r_d[:])
            nc.sync.dma_start(bc2s[:], bc2r_d[:])
            nc.sync.dma_start(iotas[:], iota_d[:])
            nc.sync.dma_start(idents[:], ident_d[:])
            nc.vector.memset(ones[:], 1.0)
            nc.vector.memset(acc1[:], 0)
            nc.vector.memset(acc2[:], 0)
            nc.vector.memset(deg[:], 0)

            # tgt cast int16 -> f32 (chunked staging)
            half = (SCOLS + 1) // 2
            for i in range(2):
                lo, hi = i * half, min((i + 1) * half, SCOLS)
                tg = strm.tile([P, half], I16)
                nc.sync.dma_start(tg[:, :hi - lo], tgtm_d[:, lo:hi])
                nc.vector.tensor_copy(out=tgtf[:, lo:hi], in_=tg[:, :hi - lo])

            # idx replicate 16 -> 128 partitions (direct from DRAM, 8x)
            for g in range(8):
                nc.sync.dma_start(idxr[16 * g:16 * (g + 1), :], idx16_d[:])

            # x cast fp8 -> bf16 (chunked staging)
            for i in range(7):
                lo, hi = i * 1792, (i + 1) * 1792
                x8 = strm.tile([P, 1792], FP8)
                nc.sync.dma_start(x8[:], xT_d[:, lo:hi])
                nc.vector.tensor_copy(out=xTt[:, lo:hi], in_=x8[:])

            # ---- DRAM: bounce + tables ----
            bounce1 = drp.tile([NBP, P], BF16)
            table1 = drp.tile([TBL, P], BF16)
            bounce2 = drp.tile([NBP, P], BF16)
            table2 = drp.tile([TBL, P], BF16)

            # ---- stage A: edge MLP over packed slots ----
            nbatch = ET // BATCH
            for b in range(nbatch):
                ex8 = strm.tile([16, BATCH], FP8)
                nc.sync.dma_start(ex8[:], exT_d[:, b * BATCH:(b + 1) * BATCH])
                ext = strm.tile([16, BATCH], BF16)
                nc.vector.tensor_copy(out=ext[:], in_=ex8[:])
                h4p = psA.tile([96, 512], F32, space="PSUM")
                for g in range(3):
                    nc.tensor.matmul(
                        out=h4p[32 * g:32 * (g + 1), :],
                        lhsT=W1s[:], rhs=ext[:, 512 * g:512 * (g + 1)],
                        start=True, stop=True)
                h4s = strm.tile([96, 512], BF16)
                nc.scalar.activation(out=h4s[:], in_=h4p[:], func=AF.Relu,
                                     bias=b1r3s[:])
                ewp = psA.tile([P, 12], F32, space="PSUM")
                for cl in range(12):
                    g, q = cl // 4, cl % 4
                    nc.tensor.matmul(
                        out=ewp[:, cl:cl + 1],
                        lhsT=h4s[32 * g:32 * (g + 1), 128 * q:128 * (q + 1)],
                        rhs=W2s[32 * g:32 * (g + 1), :],
                        start=True, stop=True)
                nc.scalar.activation(out=ewf[:, b * 12:(b + 1) * 12], in_=ewp[:],
                                     func=AF.Sigmoid, bias=b2f)

            # ---- shared window walk: yields per-window tile groups ----
            def walk(fn_window, fn_group):
                # fn_window(c, o, w, base_col) -> ctx or None
                # fn_group(ctx, t, glo, ghi, o, base_col)
                for c in range(4):
                    run_cols = int(run_len[c]) // 128
                    base_col = int(chunk_base[c]) // 128
                    o = 0
                    while o < run_cols:
                        w = min(CALL_COLS, run_cols - o)
                        ctx = fn_window(c, o, w, base_col)
                        for (t, c0, c1) in tile_cols[c]:
                            if c1 < o or c0 >= o + w:
                                continue
                            glo, ghi = max(c0, o), min(c1, o + w - 1)
                            fn_group(ctx, t, glo, ghi, o, base_col)
                        o += w

            def make_S(t, gcol):
                S = spl.tile([P, P], BF16)
                par = (t % 2) * 128
                nc.vector.tensor_scalar(
                    out=S[:], in0=iotas[:, par:par + 128],
                    scalar1=tgtf[:, gcol:gcol + 1],
                    scalar2=ewf[:, gcol:gcol + 1],
                    op0=OP.is_equal, op1=OP.mult)
                return S

            # ---- deg pass (no gathers) ----
            def deg_group(ctx, t, glo, ghi, o, base_col):
                ps = psG.tile([P, 1], F32, space="PSUM", tag="g")
                for col in range(glo, ghi + 1):
                    S = make_S(t, base_col + col)
                    nc.tensor.matmul(out=ps[:], lhsT=S[:], rhs=ones[:],
                                     start=(col == glo), stop=(col == ghi))
                nc.vector.tensor_tensor(out=deg[:, t:t + 1], in0=deg[:, t:t + 1],
                                        in1=ps[:], op=OP.add)

            walk(lambda c, o, w, b: None, deg_group)

            # dinv = 1/sqrt(deg+1)
            nc.scalar.activation(out=scr[:], in_=deg[:], func=AF.Sqrt, bias=1.0)
            nc.vector.reciprocal(out=dinv[:], in_=scr[:])

            # ---- xs = dinv * (x @ Wc1) -> bounce1 ----
            for t in range(NT):
                xp = psG.tile([P, NFIL], F32, space="PSUM", tag="g")
                nc.tensor.matmul(out=xp[:], lhsT=xTt[:, t * P:(t + 1) * P],
                                 rhs=Wc1s[:], start=True, stop=True)
                nc.scalar.activation(out=xs_loc[:, t * NFIL:(t + 1) * NFIL],
                                     in_=xp[:], func=AF.Copy,
                                     scale=dinv[:, t:t + 1])
            nc.sync.dma_start(
                bounce1[:, :NFIL].rearrange("(t p) f -> p t f", p=P),
                xs_loc[:].rearrange("p (t f) -> p t f", f=NFIL))
            nc.gpsimd.collective_compute(
                "AllGather", OP.bypass, replica_groups=[list(range(NC))],
                ins=[bounce1[:].opt()], outs=[table1[:].opt()])

            # ---- gather + S-matmul layer ----
            def layer(table, acc, RW):
                def win(c, o, w, base_col):
                    msgs = strm.tile([P, CALL_COLS, P], BF16)
                    ni = w * P
                    nc.gpsimd.dma_gather(
                        out_ap=msgs[:, :w, :], in_ap=table[c * CH:(c + 1) * CH, :],
                        idxs_ap=idxr[:, (base_col + o) * 8:(base_col + o + w) * 8],
                        num_idxs=ni, num_idxs_reg=ni,
                        elem_size=P, single_packet=False)
                    return (msgs, o)

                def grp(ctx, t, glo, ghi, o, base_col):
                    msgs, _ = ctx
                    ps = psG.tile([P, RW], F32, space="PSUM", tag="g", padded_shape=[P, NFIL])
                    for col in range(glo, ghi + 1):
                        S = make_S(t, base_col + col)
                        nc.tensor.matmul(out=ps[:], lhsT=S[:],
                                         rhs=msgs[:, col - o, :RW],
                                         start=(col == glo), stop=(col == ghi))
                    nc.vector.tensor_tensor(
                        out=acc[:, t * RW:(t + 1) * RW],
                        in0=acc[:, t * RW:(t + 1) * RW], in1=ps[:], op=OP.add)

                walk(win, grp)

            layer(table1, acc1, NFIL)

            # ---- h1s and xw2 = h1s @ Wc2 -> bounce2 ----
            for t in range(NT):
                sl = slice(t * NFIL, (t + 1) * NFIL)
                t1 = strm.tile([P, NFIL], F32)
                nc.vector.tensor_tensor(out=t1[:], in0=acc1[:, sl],
                                        in1=xs_loc[:, sl], op=OP.add)
                t2 = strm.tile([P, NFIL], F32)
                nc.scalar.activation(out=t2[:], in_=t1[:], func=AF.Copy,
                                     scale=dinv[:, t:t + 1])
                nc.vector.tensor_tensor(out=t2[:], in0=t2[:], in1=bc1s[:],
                                        op=OP.add)
                nc.vector.tensor_scalar_max(t2[:], t2[:], 0.0)
                nc.scalar.activation(out=h1s_loc[:, sl], in_=t2[:], func=AF.Copy,
                                     scale=dinv[:, t:t + 1])
                utp = psT.tile([NFIL, P], BF16, space="PSUM")
                nc.tensor.transpose(out=utp[:], in_=h1s_loc[:, sl],
                                    identity=idents[:])
                uts = strm.tile([NFIL, P], BF16)
                nc.vector.tensor_copy(out=uts[:], in_=utp[:])
                vp = psG.tile([P, CLS], F32, space="PSUM", tag="g")
                nc.tensor.matmul(out=vp[:], lhsT=uts[:], rhs=Wc2s[:],
                                 start=True, stop=True)
                nc.vector.tensor_copy(out=xw2_loc[:, t * CLS:(t + 1) * CLS],
                                      in_=vp[:])
            nc.sync.dma_start(
                bounce2[:, :CLS].rearrange("(t p) f -> p t f", p=P),
                xw2_loc[:].rearrange("p (t f) -> p t f", f=CLS))
            nc.gpsimd.collective_compute(
                "AllGather", OP.bypass, replica_groups=[list(range(NC))],
                ins=[bounce2[:].opt()], outs=[table2[:].opt()])

            layer(table2, acc2, CLS)

            # ---- final: out = log_softmax(dinv*(acc2+xw2_loc)+bc2) ----
            for t in range(NT):
                sl = slice(t * CLS, (t + 1) * CLS)
                u = strm.tile([P, CLS], F32)
                nc.vector.tensor_tensor(out=u[:], in0=acc2[:, sl],
                                        in1=xw2_loc[:, sl], op=OP.add)
                z = strm.tile([P, CLS], F32)
                nc.scalar.activation(out=z[:], in_=u[:], func=AF.Copy,
                                     scale=dinv[:, t:t + 1])
                nc.vector.tensor_tensor(out=z[:], in0=z[:], in1=bc2s[:],
                                        op=OP.add)
                nmx = strm.tile([P, 1], F32)
                nc.vector.tensor_reduce(out=nmx[:], in_=z[:],
                                        axis=mybir.AxisListType.X, op=OP.max,
                                        negate=True)
                et = strm.tile([P, CLS], F32)
                sume = strm.tile([P, 1], F32)
                nc.scalar.activation(out=et[:], in_=z[:], func=AF.Exp,
                                     bias=nmx[:], accum_out=sume[:])
                lse = strm.tile([P, 1], F32)
                nc.scalar.activation(out=lse[:], in_=sume[:], func=AF.Ln)
                res = strm.tile([P, CLS], F16)
                nc.vector.tensor_scalar(out=res[:], in0=z[:], scalar1=nmx[:],
                                        scalar2=lse[:], op0=OP.add,
                                        op1=OP.subtract)
                nc.sync.dma_start(out_d[t * P:(t + 1) * P, :], res[:])

    nc.compile()
    return nc


_last = {}


def kernel(**inputs):
    in_maps, consts, meta = _prep(**inputs)
    nc = _build(consts, meta)
    _last.update(nc=nc, in_maps=in_maps, meta=meta)
    res = run_bass_kernel_spmd(nc, in_maps, core_ids=list(range(NC)))
    _last["exec_time_ns"] = getattr(res, "exec_time_ns", None)
    out = np.zeros((N, CLS), np.float32)
    for k in range(NC):
        ok = res.results[k]["out"]          # [NBP, CLS] fp16
        out[k * NB:(k + 1) * NB] = ok[:NB].astype(np.float32)
    return out


# revision 9
# speedup vs baseline: 3.5836x; 2.2006x over previous
"""Trainium2 Bass kernel for nn_AttentionNet (GNN message passing, 2-layer GCN
with edge-MLP attention weights), 8 NeuronCores, tgt-sharded.

Strategy (v3 — lane-major layout, instruction-count minimal):
  - Core k owns target nodes [k*12500, (k+1)*12500); its nodes are sorted by
    in-degree (pads last) and laid out so partition lane = sorted position
    % 128, tile = position // 128. Tile t gets CT_t columns (cross-core max
    of the tile's top degree), so the per-node segment sum is a plain
    tensor_reduce over each tile's column range — no scatter matrices.
  - Gather indices are int16 (< 32768), so the 100352-row table is split in
    4 chunks; each window is gathered 4x with per-chunk index variants where
    foreign-chunk/pad slots point at a guaranteed-zero table row (12543,
    the last pad position of the chunk's first core).
  - Edge MLP: edge features uploaded fp8 in an 8-column interleaved layout
    [16a+f, b*128+p] so one K=128 matmul against a block-diagonal W1
    processes 8 columns; W2 stage is a broadcast-multiply + strided reduce.
  - deg = per-tile reduce of ew minus host-computed pad correction
    (pad slots have ew == sigmoid(b2) exactly, since their edge_x is 0).
  - Everything else (h1, scaling, log_softmax) is batched across all 98
    tiles in a handful of broadcast DVE ops.
"""
import sys
import numpy as np

sys.path.insert(0, "/opt/trn_rl_repo")

import ml_dtypes
import concourse.bass as bass
import concourse.tile as tile
import concourse.bacc as bacc
from concourse import mybir
from concourse.bass_utils import run_bass_kernel_spmd

NC = 8
N = 100000
NB = 12500
NBP = 12544           # padded per-core nodes (98 * 128)
NT = NBP // 128       # 98 node tiles
CH = 25088            # chunk size in global padded table (2 cores * 12544)
TBL = NC * NBP        # 100352
P = 128
NFIL, CLS = 64, 16
ZROW = 12543          # within-chunk row that is guaranteed all-zero
W_LIMIT = 48          # max gather window columns (6144 idxs)
PADLANE = NB - (NT - 1) * P   # 84: first pad lane in the last tile

F32 = mybir.dt.float32
BF16 = mybir.dt.bfloat16
F16 = mybir.dt.float16
I16 = mybir.dt.int16
U8 = mybir.dt.uint8
FP8 = mybir.dt.float8e4
AF = mybir.ActivationFunctionType
OP = mybir.AluOpType
FP8NP = mybir.dt.np(FP8)
X = mybir.AxisListType.X


def _prep(x, edge_index, edge_x, W1, b1, W2, b2, Wc1, bc1, Wc2, bc2):
    src = np.asarray(edge_index[0]).astype(np.int64)
    tgt = np.asarray(edge_index[1]).astype(np.int64)
    E = src.shape[0]
    x = np.asarray(x, np.float32)
    edge_x = np.asarray(edge_x, np.float32)

    core = tgt // NB
    local_t = tgt - core * NB

    # per-core in-degree and degree-sorted node permutation (pads last)
    degs, perms, poss = [], [], []
    ispad = (np.arange(NBP) >= NB).astype(np.int64)
    for k in range(NC):
        d = np.bincount(local_t[core == k], minlength=NBP)
        d[NB:] = 0
        perm = np.lexsort((ispad, -d))
        pos = np.empty(NBP, np.int64)
        pos[perm] = np.arange(NBP)
        degs.append(d)
        perms.append(perm)
        poss.append(pos)
    deg_sorted = np.stack([degs[k][perms[k]] for k in range(NC)])  # [NC, NBP]

    CT = deg_sorted.reshape(NC, NT, P)[:, :, 0].max(0)             # [NT]
    assert CT.max() <= 64, f"tile column count {CT.max()} > 64"
    coloff = np.zeros(NT + 1, np.int64)
    coloff[1:] = np.cumsum(CT)
    SCOLS_raw = int(coloff[NT])
    SCOLS = ((SCOLS_raw + 63) // 64) * 64
    ET = SCOLS * P

    # per-edge slot
    pos_of_tgt = np.empty(E, np.int64)
    spos = np.empty(E, np.int64)
    src_core = src // NB
    src_local = src - src_core * NB
    for k in range(NC):
        m = core == k
        pos_of_tgt[m] = poss[k][local_t[m]]
        ms = src_core == k
        spos[ms] = poss[k][src_local[ms]]
    lane = pos_of_tgt % P
    tilei = pos_of_tgt // P
    # rank within (core, tgt node)
    key = core * NBP + pos_of_tgt
    order = np.argsort(key, kind="stable")
    ks = key[order]
    newgrp = np.r_[True, ks[1:] != ks[:-1]]
    gstart = np.maximum.accumulate(np.where(newgrp, np.arange(E), 0))
    rank = np.empty(E, np.int64)
    rank[order] = np.arange(E) - gstart
    slot = (coloff[tilei] + rank) * P + lane

    srow = src_core * NBP + spos
    chunk = (src_core >> 1).astype(np.uint8)
    idxc = (srow - (src_core >> 1) * CH).astype(np.int16)

    c0 = float(1.0 / (1.0 + np.exp(-float(np.asarray(b2).reshape(-1)[0]))))

    in_maps = []
    for k in range(NC):
        m = core == k
        sl = slot[m]
        ex2 = np.zeros((ET, 16), np.float32)
        ex2[sl] = edge_x[m]
        # stage-A interleaved layout [16a+f, b*128+p], slot=(8b+a)*128+p
        ex4 = ex2.reshape(ET // 1024, 8, P, 16)           # [b, a, p, f]
        exT8 = ex4.transpose(1, 3, 0, 2).reshape(P, ET // 8).astype(FP8NP)
        iw = np.zeros(ET, np.int16)
        iw[sl] = idxc[m]
        cw = np.full(ET, 255, np.uint8)
        cw[sl] = chunk[m]
        xt = np.zeros((P, NBP), FP8NP)
        valid = perms[k] < NB
        xt[:, valid.nonzero()[0]] = x[k * NB + perms[k][valid]].T.astype(FP8NP)
        npc = ((CT[None, :] - degs[k][perms[k]].reshape(NT, P).T)
               .astype(np.float32) * c0)                   # [P(lane), NT]
        in_maps.append({
            "exT8": np.ascontiguousarray(exT8),
            "idx16": np.ascontiguousarray(iw.reshape(ET // 16, 16).T),
            "chunk8": np.ascontiguousarray(cw.reshape(ET // 16, 16).T),
            "xT": xt,
            "npc": npc.astype(ml_dtypes.bfloat16),
        })

    W1 = np.asarray(W1, np.float32)
    W1blk = np.zeros((P, 256), np.float32)
    for a in range(8):
        W1blk[16 * a:16 * (a + 1), 32 * a:32 * (a + 1)] = W1
    w2row = np.tile(np.asarray(W2, np.float32).reshape(1, 32), (P, 1))
    Wc2 = np.asarray(Wc2, np.float32)
    wc2f = np.tile(Wc2.T.reshape(1, CLS * NFIL), (P, 1))   # [P, k*64+f]

    consts = dict(
        W1blk=W1blk.astype(ml_dtypes.bfloat16),
        b1r8=np.tile(np.asarray(b1, np.float32), 8)[None, :].repeat(P, 0),
        w2row=w2row.astype(ml_dtypes.bfloat16),
        Wc1=np.asarray(Wc1, np.float32).astype(ml_dtypes.bfloat16),
        wc2f=wc2f.astype(ml_dtypes.bfloat16),
        bc1r=np.tile(np.asarray(bc1, np.float32)[None, :], (P, 1)),
        bc2r=np.tile(np.asarray(bc2, np.float32)[None, :], (P, 1)),
        b2f=float(np.asarray(b2, np.float32).reshape(-1)[0]),
        padmask=(np.arange(P) < PADLANE).astype(np.float32)[:, None],
    )
    meta = dict(CT=CT, coloff=coloff, SCOLS=SCOLS, SCOLS_raw=SCOLS_raw, ET=ET,
                perms=perms)
    return in_maps, consts, meta


def _build(consts, meta, parts=frozenset({"stageA", "deg", "ag1", "l1", "h1",
                                          "ag2", "l2"})):
    CT, coloff = meta["CT"], meta["coloff"]
    SCOLS, ET = meta["SCOLS"], meta["ET"]
    nc = bacc.Bacc("TRN2", target_bir_lowering=False, debug=False, num_devices=NC)

    exT8_d = nc.dram_tensor("exT8", [P, ET // 8], FP8, kind="ExternalInput")
    idx16_d = nc.dram_tensor("idx16", [16, ET // 16], I16, kind="ExternalInput")
    chunk8_d = nc.dram_tensor("chunk8", [16, ET // 16], U8, kind="ExternalInput")
    xT_d = nc.dram_tensor("xT", [P, NBP], FP8, kind="ExternalInput")
    npc_d = nc.dram_tensor("npc", [P, NT], BF16, kind="ExternalInput")
    out_d = nc.dram_tensor("out", [NBP, CLS], F16, kind="ExternalOutput")

    W1blk_d = nc.inline_tensor(consts["W1blk"], "W1blk")
    b1r8_d = nc.inline_tensor(consts["b1r8"], "b1r8")
    w2row_d = nc.inline_tensor(consts["w2row"], "w2row")
    Wc1_d = nc.inline_tensor(consts["Wc1"], "Wc1")
    wc2f_d = nc.inline_tensor(consts["wc2f"], "wc2f")
    bc1r_d = nc.inline_tensor(consts["bc1r"], "bc1r")
    bc2r_d = nc.inline_tensor(consts["bc2r"], "bc2r")
    padmask_d = nc.inline_tensor(consts["padmask"], "padmask")
    b2f = consts["b2f"]

    # greedy-pack whole tiles into gather windows of <= W_LIMIT columns
    windows = []  # (c0, c1, [(t, lo, hi)]) column ranges, hi exclusive
    cur = []
    for t in range(NT):
        lo, hi = int(coloff[t]), int(coloff[t + 1])
        if hi == lo:
            continue
        if cur and hi - cur[0][1] > W_LIMIT:
            windows.append((cur[0][1], cur[-1][2], cur))
            cur = []
        cur.append((t, lo, hi))
    if cur:
        windows.append((cur[0][1], cur[-1][2], cur))
    WMAX = max(c1 - c0 for c0, c1, _ in windows)
    ETC = ET // 128   # idx columns in [128, ETC] reshaped view

    with tile.TileContext(nc) as tc:
        with (
            tc.tile_pool(name="persist", bufs=1) as pers,
            tc.tile_pool(name="psA", bufs=3, space="PSUM") as psA,
            tc.tile_pool(name="psX", bufs=2, space="PSUM") as psX,
            tc.tile_pool(name="dram", bufs=1, space="DRAM") as drp,
        ):
            # ---- persistent tiles ----
            ewpre = pers.tile([P, SCOLS], F32)
            ewf = pers.tile([P, SCOLS], BF16)
            xs = pers.tile([P, NT * NFIL], BF16)
            h1s = pers.tile([P, NT * NFIL], BF16)
            acc = pers.tile([P, NT * NFIL], F32)
            zpre = pers.tile([P, NT * CLS], F32)
            deg = pers.tile([P, NT], F32)
            dinv = pers.tile([P, NT], F32)
            scr = pers.tile([P, NT], F32)
            npcs = pers.tile([P, NT], BF16)
            W1blks = pers.tile([P, 256], BF16)
            b1r8s = pers.tile([P, 256], F32)
            w2rows = pers.tile([P, 32], BF16)
            Wc1s = pers.tile([P, NFIL], BF16)
            wc2fs = pers.tile([P, CLS * NFIL], BF16)
            bc1s = pers.tile([P, NFIL], F32)
            bc2s = pers.tile([P, CLS], F32)
            padm = pers.tile([P, 1], F32)

            nc.sync.dma_start(W1blks[:], W1blk_d[:])
            nc.sync.dma_start(b1r8s[:], b1r8_d[:])
            nc.sync.dma_start(w2rows[:], w2row_d[:])
            nc.sync.dma_start(Wc1s[:], Wc1_d[:])
            nc.sync.dma_start(wc2fs[:], wc2f_d[:])
            nc.sync.dma_start(bc1s[:], bc1r_d[:])
            nc.sync.dma_start(bc2s[:], bc2r_d[:])
            nc.sync.dma_start(npcs[:], npc_d[:])
            nc.sync.dma_start(padm[:], padmask_d[:])

            # ---- DRAM: replicated idx variants, bounce + tables ----
            idxw = [drp.tile([P, ET // 16], I16, name=f"idxw{c}") for c in range(4)]
            bounce1 = drp.tile([NBP, P], BF16)
            table1 = drp.tile([TBL, P], BF16)
            bounce2 = drp.tile([NBP, P], BF16)
            table2 = drp.tile([TBL, P], BF16)

            with tc.tile_pool(name="early", bufs=1) as erl:
                # build 4 idx variants: idx_c = 12543 + (chunk==c)*(idx16-12543)
                # processed in a [128, ETC] reshaped view of the [16, ET//16] wrap
                i16b = erl.tile([P, ETC], I16)
                ch8b = erl.tile([P, ETC], U8)
                nc.sync.dma_start(
                    i16b[:], idx16_d[:].rearrange("r (s c) -> (r s) c", s=8))
                nc.sync.dma_start(
                    ch8b[:], chunk8_d[:].rearrange("r (s c) -> (r s) c", s=8))
                dif = erl.tile([P, ETC], I16)
                nc.vector.tensor_scalar(out=dif[:], in0=i16b[:],
                                        scalar1=float(ZROW),
                                        scalar2=None, op0=OP.subtract)
                for c in range(4):
                    sel = erl.tile([P, ETC], I16, tag="sel", bufs=2)
                    nc.vector.tensor_scalar(out=sel[:], in0=ch8b[:],
                                            scalar1=float(c),
                                            scalar2=None, op0=OP.is_equal)
                    nc.vector.tensor_tensor(out=sel[:], in0=sel[:], in1=dif[:],
                                            op=OP.mult)
                    nc.vector.tensor_scalar(out=sel[:], in0=sel[:],
                                            scalar1=float(ZROW),
                                            scalar2=None, op0=OP.add)
                    for g in range(8):
                        nc.sync.dma_start(
                            idxw[c][16 * g:16 * (g + 1), :].rearrange(
                                "r (s c2) -> (r s) c2", s=8),
                            sel[:])

                # ---- stage A: edge MLP ----
                if "stageA" in parts:
                    QC = SCOLS // 8            # columns per mega-chunk (mult of 8)
                    with tc.tile_pool(name="sa", bufs=2) as sa:
                        for q in range(8):
                            ex8 = sa.tile([P, QC * 16], FP8, tag="ex8")
                            nc.sync.dma_start(
                                ex8[:],
                                exT8_d[:, q * QC * 16:(q + 1) * QC * 16])
                            exb = sa.tile([P, QC * 16], BF16, tag="exb")
                            nc.vector.tensor_copy(out=exb[:], in_=ex8[:])
                            h8s = sa.tile([P, QC * 32], BF16, tag="h8s", bufs=1)
                            ng = QC // 8
                            for g in range(0, ng, 2):
                                ps = psA.tile([P, 512], F32, space="PSUM", tag="a")
                                npair = min(2, ng - g)
                                for j in range(npair):
                                    nc.tensor.matmul(
                                        out=ps[:, 256 * j:256 * (j + 1)],
                                        lhsT=exb[:, (g + j) * P:(g + j + 1) * P],
                                        rhs=W1blks[:], start=True, stop=True)
                                w = npair * 256
                                nc.scalar.activation(
                                    out=h8s[:, g * 256:g * 256 + w],
                                    in_=ps[:, :w], func=AF.Relu,
                                    bias=b1r8s[:, :1])
                            hw = sa.tile([P, QC * 32], BF16, tag="hw", bufs=1)
                            nc.vector.tensor_tensor(
                                out=hw[:].rearrange("p (c i) -> p c i", i=32),
                                in0=h8s[:].rearrange("p (c i) -> p c i", i=32),
                                in1=w2rows[:].unsqueeze(1).to_broadcast(
                                    [P, QC, 32]),
                                op=OP.mult)
                            nc.vector.tensor_reduce(
                                out=ewpre[:, q * QC:(q + 1) * QC],
                                in_=hw[:].rearrange("p (c i) -> p c i", i=32),
                                axis=X, op=OP.add)
                    nc.scalar.activation(out=ewf[:], in_=ewpre[:], func=AF.Sigmoid,
                                         bias=b2f)
                else:
                    nc.vector.memset(ewf[:], 0.25)

                # ---- deg / dinv ----
                if "deg" in parts:
                    for t in range(NT):
                        lo, hi = int(coloff[t]), int(coloff[t + 1])
                        if hi == lo:
                            nc.vector.memset(deg[:, t:t + 1], 0)
                            continue
                        nc.vector.tensor_reduce(out=deg[:, t:t + 1],
                                                in_=ewf[:, lo:hi], axis=X,
                                                op=OP.add)
                    nc.vector.tensor_tensor(out=deg[:], in0=deg[:], in1=npcs[:],
                                            op=OP.subtract)
                else:
                    nc.vector.memset(deg[:], 3.0)
                nc.scalar.activation(out=scr[:], in_=deg[:], func=AF.Sqrt, bias=1.0)
                nc.vector.reciprocal(out=dinv[:], in_=scr[:])

                # ---- xs = dinv * (x @ Wc1) ----
                xT8 = erl.tile([P, NBP], FP8)
                nc.sync.dma_start(xT8[:], xT_d[:])
                xTt = erl.tile([P, NBP], BF16)
                nc.vector.tensor_copy(out=xTt[:], in_=xT8[:])
                for t0 in range(0, NT, 8):
                    nt8 = min(8, NT - t0)
                    ps = psX.tile([P, 512], F32, space="PSUM", tag="x")
                    for j in range(nt8):
                        nc.tensor.matmul(
                            out=ps[:, j * NFIL:(j + 1) * NFIL],
                            lhsT=xTt[:, (t0 + j) * P:(t0 + j + 1) * P],
                            rhs=Wc1s[:], start=True, stop=True)
                    nc.vector.tensor_copy(
                        out=xs[:, t0 * NFIL:(t0 + nt8) * NFIL],
                        in_=ps[:, :nt8 * NFIL])
                nc.vector.tensor_tensor(
                    out=xs[:].rearrange("p (t f) -> p t f", f=NFIL),
                    in0=xs[:].rearrange("p (t f) -> p t f", f=NFIL),
                    in1=dinv[:].unsqueeze(2).to_broadcast([P, NT, NFIL]),
                    op=OP.mult)

            nc.sync.dma_start(
                bounce1[:, :NFIL].rearrange("(t p) f -> p t f", p=P),
                xs[:].rearrange("p (t f) -> p t f", f=NFIL))
            if "ag1" in parts:
                nc.gpsimd.collective_compute(
                    "AllGather", OP.bypass, replica_groups=[list(range(NC))],
                    ins=[bounce1[:].opt()], outs=[table1[:].opt()])

            with tc.tile_pool(name="lay", bufs=2) as lay:
                # ---- gather + reduce layer ----
                def layer(table, accT):
                    for (wc0, wc1, tl) in windows:
                        w = wc1 - wc0
                        mA = lay.tile([P, WMAX, P], BF16, tag="mA")
                        for c in range(4):
                            idxt = lay.tile([P, WMAX * 8], I16, tag="idx", bufs=4)
                            nc.sync.dma_start(idxt[:, :w * 8],
                                              idxw[c][:, wc0 * 8:wc1 * 8])
                            tgt_m = mA if c == 0 else lay.tile(
                                [P, WMAX, P], BF16, tag="mB", bufs=3)
                            nc.gpsimd.dma_gather(
                                out_ap=tgt_m[:, :w, :],
                                in_ap=table[c * CH:(c + 1) * CH, :],
                                idxs_ap=idxt[:, :w * 8], num_idxs=w * P,
                                num_idxs_reg=w * P, elem_size=P,
                                single_packet=False)
                            if c > 0:
                                nc.vector.tensor_tensor(
                                    out=mA[:, :w, :NFIL], in0=mA[:, :w, :NFIL],
                                    in1=tgt_m[:, :w, :NFIL], op=OP.add)
                        sc = lay.tile([P, WMAX, NFIL], BF16, tag="sc")
                        nc.vector.tensor_tensor(
                            out=sc[:, :w, :], in0=mA[:, :w, :NFIL],
                            in1=ewf[:, wc0:wc1].unsqueeze(2).to_broadcast(
                                [P, w, NFIL]),
                            op=OP.mult)
                        for (t, lo, hi) in tl:
                            nc.vector.tensor_reduce(
                                out=accT[:, t * NFIL:(t + 1) * NFIL],
                                in_=sc[:, lo - wc0:hi - wc0, :].transpose(
                                    [0, 2, 1]),
                                axis=X, op=OP.add)
                    # tiles with zero columns never get written: zero them
                    for t in range(NT):
                        if coloff[t + 1] == coloff[t]:
                            nc.vector.memset(accT[:, t * NFIL:(t + 1) * NFIL], 0)

                if "l1" in parts:
                    layer(table1, acc)
                else:
                    nc.vector.memset(acc[:], 0)

                # ---- h1s = dinv*relu(dinv*(acc+xs)+bc1) ----
                nc.vector.tensor_tensor(out=acc[:], in0=acc[:], in1=xs[:],
                                        op=OP.add)
                nc.vector.tensor_tensor(
                    out=acc[:].rearrange("p (t f) -> p t f", f=NFIL),
                    in0=acc[:].rearrange("p (t f) -> p t f", f=NFIL),
                    in1=dinv[:].unsqueeze(2).to_broadcast([P, NT, NFIL]),
                    op=OP.mult)
                nc.vector.tensor_tensor(
                    out=acc[:].rearrange("p (t f) -> p t f", f=NFIL),
                    in0=acc[:].rearrange("p (t f) -> p t f", f=NFIL),
                    in1=bc1s[:].unsqueeze(1).to_broadcast([P, NT, NFIL]),
                    op=OP.add)
                nc.vector.tensor_scalar_max(acc[:], acc[:], 0.0)
                nc.vector.tensor_tensor(
                    out=h1s[:].rearrange("p (t f) -> p t f", f=NFIL),
                    in0=acc[:].rearrange("p (t f) -> p t f", f=NFIL),
                    in1=dinv[:].unsqueeze(2).to_broadcast([P, NT, NFIL]),
                    op=OP.mult)
                # zero pad-node rows (positions 12500..12543): zero-row trick
                nc.vector.tensor_scalar(
                    out=h1s[:, (NT - 1) * NFIL:], in0=h1s[:, (NT - 1) * NFIL:],
                    scalar1=padm[:], scalar2=None, op0=OP.mult)

                nc.sync.dma_start(
                    bounce2[:, :NFIL].rearrange("(t p) f -> p t f", p=P),
                    h1s[:].rearrange("p (t f) -> p t f", f=NFIL))
                if "ag2" in parts:
                    nc.gpsimd.collective_compute(
                        "AllGather", OP.bypass, replica_groups=[list(range(NC))],
                        ins=[bounce2[:].opt()], outs=[table2[:].opt()])

                nc.vector.memset(acc[:], 0)
                if "l2" in parts:
                    layer(table2, acc)

            # ---- final: log_softmax(dinv*((acc+h1s) @ Wc2) + bc2) ----
            nc.vector.tensor_tensor(out=acc[:], in0=acc[:], in1=h1s[:], op=OP.add)
            with tc.tile_pool(name="fin", bufs=1) as fin:
                for t0 in range(0, NT, 8):
                    nt8 = min(8, NT - t0)
                    tmp = fin.tile([P, 8 * CLS * NFIL], F32, tag="tmp")
                    nc.vector.tensor_tensor(
                        out=tmp[:, :nt8 * CLS * NFIL].rearrange(
                            "p (t k f) -> p t k f", k=CLS, f=NFIL),
                        in0=acc[:, t0 * NFIL:(t0 + nt8) * NFIL].rearrange(
                            "p (t f) -> p t f", f=NFIL).unsqueeze(2)
                            .to_broadcast([P, nt8, CLS, NFIL]),
                        in1=wc2fs[:].rearrange("p (k f) -> p k f", f=NFIL)
                            .unsqueeze(1).to_broadcast([P, nt8, CLS, NFIL]),
                        op=OP.mult)
                    nc.vector.tensor_reduce(
                        out=zpre[:, t0 * CLS:(t0 + nt8) * CLS],
                        in_=tmp[:, :nt8 * CLS * NFIL].rearrange(
                            "p (c f) -> p c f", f=NFIL),
                        axis=X, op=OP.add)
                nc.vector.tensor_tensor(
                    out=zpre[:].rearrange("p (t k) -> p t k", k=CLS),
                    in0=zpre[:].rearrange("p (t k) -> p t k", k=CLS),
                    in1=dinv[:].unsqueeze(2).to_broadcast([P, NT, CLS]),
                    op=OP.mult)
                nc.vector.tensor_tensor(
                    out=zpre[:].rearrange("p (t k) -> p t k", k=CLS),
                    in0=zpre[:].rearrange("p (t k) -> p t k", k=CLS),
                    in1=bc2s[:].unsqueeze(1).to_broadcast([P, NT, CLS]),
                    op=OP.add)
                nmx = fin.tile([P, NT], F32)
                nc.vector.tensor_reduce(
                    out=nmx[:], in_=zpre[:].rearrange("p (t k) -> p t k", k=CLS),
                    axis=X, op=OP.max, negate=True)
                nc.vector.tensor_tensor(
                    out=zpre[:].rearrange("p (t k) -> p t k", k=CLS),
                    in0=zpre[:].rearrange("p (t k) -> p t k", k=CLS),
                    in1=nmx[:].unsqueeze(2).to_broadcast([P, NT, CLS]), op=OP.add)
                et = fin.tile([P, NT * CLS], F32)
                nc.scalar.activation(out=et[:], in_=zpre[:], func=AF.Exp)
                sume = fin.tile([P, NT], F32)
                nc.vector.tensor_reduce(
                    out=sume[:], in_=et[:].rearrange("p (t k) -> p t k", k=CLS),
                    axis=X, op=OP.add)
                lse = fin.tile([P, NT], F32)
                nc.scalar.activation(out=lse[:], in_=sume[:], func=AF.Ln)
                res = fin.tile([P, NT * CLS], F16)
                nc.vector.tensor_tensor(
                    out=res[:].rearrange("p (t k) -> p t k", k=CLS),
                    in0=zpre[:].rearrange("p (t k) -> p t k", k=CLS),
                    in1=lse[:].unsqueeze(2).to_broadcast([P, NT, CLS]),
                    op=OP.subtract)
                nc.sync.dma_start(
                    out_d[:].rearrange("(t p) f -> p t f", p=P),
                    res[:].rearrange("p (t f) -> p t f", f=CLS))

    nc.compile()
    return nc


_last = {}


def kernel(**inputs):
    in_maps, consts, meta = _prep(**inputs)
    nc = _build(consts, meta)
    _last.update(nc=nc, in_maps=in_maps, meta=meta)
    res = run_bass_kernel_spmd(nc, in_maps, core_ids=list(range(NC)))
    _last["exec_time_ns"] = getattr(res, "exec_time_ns", None)
    out = np.zeros((N, CLS), np.float32)
    for k in range(NC):
        ok = res.results[k]["out"].astype(np.float32)   # [NBP, CLS] sorted order
        perm = meta["perms"][k]
        valid = perm < NB
        out[k * NB + perm[valid]] = ok[valid.nonzero()[0]]
    return out


# revision 11
# speedup vs baseline: 3.8703x; 1.0800x over previous
"""Trainium2 Bass kernel for nn_AttentionNet (GNN message passing, 2-layer GCN
with edge-MLP attention weights), 8 NeuronCores, tgt-sharded.

Strategy (v3 — lane-major layout, instruction-count minimal):
  - Core k owns target nodes [k*12500, (k+1)*12500); its nodes are sorted by
    in-degree (pads last) and laid out so partition lane = sorted position
    % 128, tile = position // 128. Tile t gets CT_t columns (cross-core max
    of the tile's top degree), so the per-node segment sum is a plain
    tensor_reduce over each tile's column range — no scatter matrices.
  - Gather indices are int16 (< 32768), so the 100352-row table is split in
    4 chunks; each window is gathered 4x with per-chunk index variants where
    foreign-chunk/pad slots point at a guaranteed-zero table row (12543,
    the last pad position of the chunk's first core).
  - Edge MLP: edge features uploaded fp8 in an 8-column interleaved layout
    [16a+f, b*128+p] so one K=128 matmul against a block-diagonal W1
    processes 8 columns; W2 stage is a broadcast-multiply + strided reduce.
  - deg = per-tile reduce of ew minus host-computed pad correction
    (pad slots have ew == sigmoid(b2) exactly, since their edge_x is 0).
  - Everything else (h1, scaling, log_softmax) is batched across all 98
    tiles in a handful of broadcast DVE ops.
"""
import sys
import numpy as np

sys.path.insert(0, "/opt/trn_rl_repo")

import ml_dtypes
import concourse.bass as bass
import concourse.tile as tile
import concourse.bacc as bacc
from concourse import mybir
from concourse.bass_utils import run_bass_kernel_spmd

NC = 8
N = 100000
NB = 12500
NBP = 12544           # padded per-core nodes (98 * 128)
NT = NBP // 128       # 98 node tiles
CH = 25088            # chunk size in global padded table (2 cores * 12544)
TBL = NC * NBP        # 100352
P = 128
NFIL, CLS = 64, 16
ZROW = 12543          # within-chunk row that is guaranteed all-zero
W_LIMIT = 48          # max gather window columns (6144 idxs)
PADLANE = NB - (NT - 1) * P   # 84: first pad lane in the last tile

F32 = mybir.dt.float32
BF16 = mybir.dt.bfloat16
F16 = mybir.dt.float16
I16 = mybir.dt.int16
U8 = mybir.dt.uint8
FP8 = mybir.dt.float8e4
AF = mybir.ActivationFunctionType
OP = mybir.AluOpType
FP8NP = mybir.dt.np(FP8)
X = mybir.AxisListType.X


def _prep(x, edge_index, edge_x, W1, b1, W2, b2, Wc1, bc1, Wc2, bc2):
    src = np.asarray(edge_index[0]).astype(np.int64)
    tgt = np.asarray(edge_index[1]).astype(np.int64)
    E = src.shape[0]
    x = np.asarray(x, np.float32)
    edge_x = np.asarray(edge_x, np.float32)

    core = tgt // NB
    local_t = tgt - core * NB

    # per-core in-degree and degree-sorted node permutation (pads last)
    degs, perms, poss = [], [], []
    ispad = (np.arange(NBP) >= NB).astype(np.int64)
    for k in range(NC):
        d = np.bincount(local_t[core == k], minlength=NBP)
        d[NB:] = 0
        perm = np.lexsort((ispad, -d))
        pos = np.empty(NBP, np.int64)
        pos[perm] = np.arange(NBP)
        degs.append(d)
        perms.append(perm)
        poss.append(pos)
    deg_sorted = np.stack([degs[k][perms[k]] for k in range(NC)])  # [NC, NBP]

    CT = deg_sorted.reshape(NC, NT, P)[:, :, 0].max(0)             # [NT]
    assert CT.max() <= 64, f"tile column count {CT.max()} > 64"
    coloff = np.zeros(NT + 1, np.int64)
    coloff[1:] = np.cumsum(CT)
    SCOLS_raw = int(coloff[NT])
    SCOLS = ((SCOLS_raw + 63) // 64) * 64
    ET = SCOLS * P

    # per-edge slot
    pos_of_tgt = np.empty(E, np.int64)
    spos = np.empty(E, np.int64)
    src_core = src // NB
    src_local = src - src_core * NB
    for k in range(NC):
        m = core == k
        pos_of_tgt[m] = poss[k][local_t[m]]
        ms = src_core == k
        spos[ms] = poss[k][src_local[ms]]
    lane = pos_of_tgt % P
    tilei = pos_of_tgt // P
    # rank within (core, tgt node)
    key = core * NBP + pos_of_tgt
    order = np.argsort(key, kind="stable")
    ks = key[order]
    newgrp = np.r_[True, ks[1:] != ks[:-1]]
    gstart = np.maximum.accumulate(np.where(newgrp, np.arange(E), 0))
    rank = np.empty(E, np.int64)
    rank[order] = np.arange(E) - gstart
    slot = (coloff[tilei] + rank) * P + lane

    srow = src_core * NBP + spos
    idxc = (srow >> 2).astype(np.int16)        # 4-node pack row, < 25088
    sub4 = (srow & 3).astype(np.uint8)

    c0 = float(1.0 / (1.0 + np.exp(-float(np.asarray(b2).reshape(-1)[0]))))

    in_maps = []
    for k in range(NC):
        m = core == k
        sl = slot[m]
        ex2 = np.zeros((ET, 16), np.float32)
        ex2[sl] = edge_x[m]
        # stage-A interleaved layout [16a+f, b*128+p], slot=(8b+a)*128+p
        ex4 = ex2.reshape(ET // 1024, 8, P, 16)           # [b, a, p, f]
        exT8 = ex4.transpose(1, 3, 0, 2).reshape(P, ET // 8).astype(FP8NP)
        iw = np.zeros(ET, np.int16)
        iw[sl] = idxc[m]
        cw = np.full(ET, 255, np.uint8)
        cw[sl] = sub4[m]
        xt = np.zeros((P, NBP), FP8NP)
        valid = perms[k] < NB
        xt[:, valid.nonzero()[0]] = x[k * NB + perms[k][valid]].T.astype(FP8NP)
        npc = ((CT[None, :] - degs[k][perms[k]].reshape(NT, P).T)
               .astype(np.float32) * c0)                   # [P(lane), NT]
        in_maps.append({
            "exT8": np.ascontiguousarray(exT8),
            "idx16": np.ascontiguousarray(iw.reshape(ET // 16, 16).T),
            "s4": np.ascontiguousarray(cw.reshape(SCOLS, P).T),
            "xT": xt,
            "npc": npc.astype(ml_dtypes.bfloat16),
        })

    W1 = np.asarray(W1, np.float32)
    W1blk = np.zeros((P, 256), np.float32)
    for a in range(8):
        W1blk[16 * a:16 * (a + 1), 32 * a:32 * (a + 1)] = W1
    w2row = np.tile(np.asarray(W2, np.float32).reshape(1, 32), (P, 1))
    Wc2 = np.asarray(Wc2, np.float32)
    wc2f = np.tile(Wc2.T.reshape(1, CLS * NFIL), (P, 1))   # [P, k*64+f]

    consts = dict(
        W1blk=W1blk.astype(ml_dtypes.bfloat16),
        b1r8=np.tile(np.asarray(b1, np.float32), 8)[None, :].repeat(P, 0),
        w2row=w2row.astype(ml_dtypes.bfloat16),
        Wc1=np.asarray(Wc1, np.float32).astype(ml_dtypes.bfloat16),
        wc2f=wc2f.astype(ml_dtypes.bfloat16),
        bc1r=np.tile(np.asarray(bc1, np.float32)[None, :], (P, 1)),
        bc2r=np.tile(np.asarray(bc2, np.float32)[None, :], (P, 1)),
        b2f=float(np.asarray(b2, np.float32).reshape(-1)[0]),
    )
    meta = dict(CT=CT, coloff=coloff, SCOLS=SCOLS, SCOLS_raw=SCOLS_raw, ET=ET,
                perms=perms)
    return in_maps, consts, meta


def _build(consts, meta, parts=frozenset({"stageA", "deg", "ag1", "l1", "h1",
                                          "ag2", "l2"})):
    CT, coloff = meta["CT"], meta["coloff"]
    SCOLS, ET = meta["SCOLS"], meta["ET"]
    nc = bacc.Bacc("TRN2", target_bir_lowering=False, debug=False, num_devices=NC)

    exT8_d = nc.dram_tensor("exT8", [P, ET // 8], FP8, kind="ExternalInput")
    idx16_d = nc.dram_tensor("idx16", [16, ET // 16], I16, kind="ExternalInput")
    s4_d = nc.dram_tensor("s4", [P, SCOLS], U8, kind="ExternalInput")
    xT_d = nc.dram_tensor("xT", [P, NBP], FP8, kind="ExternalInput")
    npc_d = nc.dram_tensor("npc", [P, NT], BF16, kind="ExternalInput")
    out_d = nc.dram_tensor("out", [NBP, CLS], F16, kind="ExternalOutput")

    W1blk_d = nc.inline_tensor(consts["W1blk"], "W1blk")
    b1r8_d = nc.inline_tensor(consts["b1r8"], "b1r8")
    w2row_d = nc.inline_tensor(consts["w2row"], "w2row")
    Wc1_d = nc.inline_tensor(consts["Wc1"], "Wc1")
    wc2f_d = nc.inline_tensor(consts["wc2f"], "wc2f")
    bc1r_d = nc.inline_tensor(consts["bc1r"], "bc1r")
    bc2r_d = nc.inline_tensor(consts["bc2r"], "bc2r")
    b2f = consts["b2f"]

    # greedy-pack whole tiles into gather windows of <= W_LIMIT columns
    windows = []  # (c0, c1, [(t, lo, hi)]) column ranges, hi exclusive
    cur = []
    for t in range(NT):
        lo, hi = int(coloff[t]), int(coloff[t + 1])
        if hi == lo:
            continue
        if cur and hi - cur[0][1] > W_LIMIT:
            windows.append((cur[0][1], cur[-1][2], cur))
            cur = []
        cur.append((t, lo, hi))
    if cur:
        windows.append((cur[0][1], cur[-1][2], cur))
    WMAX = max(c1 - c0 for c0, c1, _ in windows)
    ETC = ET // 128   # idx columns in [128, ETC] reshaped view

    with tile.TileContext(nc) as tc:
        with (
            tc.tile_pool(name="persist", bufs=1) as pers,
            tc.tile_pool(name="psA", bufs=3, space="PSUM") as psA,
            tc.tile_pool(name="psX", bufs=2, space="PSUM") as psX,
            tc.tile_pool(name="dram", bufs=1, space="DRAM") as drp,
        ):
            # ---- persistent tiles ----
            ewpre = pers.tile([P, SCOLS], F32)
            ewf = pers.tile([P, SCOLS], BF16)
            xs = pers.tile([P, NT * NFIL], BF16)
            h1s = pers.tile([P, NT * NFIL], BF16)
            acc = pers.tile([P, NT * NFIL], F32)
            zpre = pers.tile([P, NT * CLS], F32)
            deg = pers.tile([P, NT], F32)
            dinv = pers.tile([P, NT], F32)
            scr = pers.tile([P, NT], F32)
            npcs = pers.tile([P, NT], BF16)
            W1blks = pers.tile([P, 256], BF16)
            b1r8s = pers.tile([P, 256], F32)
            w2rows = pers.tile([P, 32], BF16)
            Wc1s = pers.tile([P, NFIL], BF16)
            wc2fs = pers.tile([P, CLS * NFIL], BF16)
            bc1s = pers.tile([P, NFIL], F32)
            bc2s = pers.tile([P, CLS], F32)
            esel = pers.tile([P, SCOLS * 4], BF16)
            s4s = pers.tile([P, SCOLS], U8)

            nc.sync.dma_start(W1blks[:], W1blk_d[:])
            nc.sync.dma_start(b1r8s[:], b1r8_d[:])
            nc.sync.dma_start(w2rows[:], w2row_d[:])
            nc.sync.dma_start(Wc1s[:], Wc1_d[:])
            nc.sync.dma_start(wc2fs[:], wc2f_d[:])
            nc.sync.dma_start(bc1s[:], bc1r_d[:])
            nc.sync.dma_start(bc2s[:], bc2r_d[:])
            nc.sync.dma_start(npcs[:], npc_d[:])
            nc.sync.dma_start(s4s[:], s4_d[:])

            # ---- DRAM: replicated idx variants, bounce + tables ----
            idxw = drp.tile([P, ET // 16], I16)
            bounce1 = drp.tile([NBP, NFIL], BF16)
            table1 = drp.tile([TBL // 4, 256], BF16)
            bounce2 = drp.tile([NBP, NFIL], BF16)
            table2 = drp.tile([TBL // 4, 256], BF16)

            with tc.tile_pool(name="early", bufs=1) as erl:
                # replicate gather idx to 128 partitions (DRAM staging)
                i16s = erl.tile([16, ET // 16], I16)
                nc.sync.dma_start(i16s[:], idx16_d[:])
                for g in range(8):
                    nc.sync.dma_start(idxw[16 * g:16 * (g + 1), :], i16s[:])

                # ---- stage A: edge MLP ----
                if "stageA" in parts:
                    QC = SCOLS // 8            # columns per mega-chunk (mult of 8)
                    with tc.tile_pool(name="sa", bufs=2) as sa:
                        for q in range(8):
                            ex8 = sa.tile([P, QC * 16], FP8, tag="ex8")
                            nc.sync.dma_start(
                                ex8[:],
                                exT8_d[:, q * QC * 16:(q + 1) * QC * 16])
                            exb = sa.tile([P, QC * 16], BF16, tag="exb")
                            nc.vector.tensor_copy(out=exb[:], in_=ex8[:])
                            h8s = sa.tile([P, QC * 32], BF16, tag="h8s", bufs=1)
                            ng = QC // 8
                            for g in range(0, ng, 2):
                                ps = psA.tile([P, 512], F32, space="PSUM", tag="a")
                                npair = min(2, ng - g)
                                for j in range(npair):
                                    nc.tensor.matmul(
                                        out=ps[:, 256 * j:256 * (j + 1)],
                                        lhsT=exb[:, (g + j) * P:(g + j + 1) * P],
                                        rhs=W1blks[:], start=True, stop=True)
                                w = npair * 256
                                nc.scalar.activation(
                                    out=h8s[:, g * 256:g * 256 + w],
                                    in_=ps[:, :w], func=AF.Relu,
                                    bias=b1r8s[:, :1])
                            hw = sa.tile([P, QC * 32], BF16, tag="hw", bufs=1)
                            nc.vector.tensor_tensor(
                                out=hw[:].rearrange("p (c i) -> p c i", i=32),
                                in0=h8s[:].rearrange("p (c i) -> p c i", i=32),
                                in1=w2rows[:].unsqueeze(1).to_broadcast(
                                    [P, QC, 32]),
                                op=OP.mult)
                            nc.vector.tensor_reduce(
                                out=ewpre[:, q * QC:(q + 1) * QC],
                                in_=hw[:].rearrange("p (c i) -> p c i", i=32),
                                axis=X, op=OP.add)
                    nc.scalar.activation(out=ewf[:], in_=ewpre[:], func=AF.Sigmoid,
                                         bias=b2f)
                else:
                    nc.vector.memset(ewf[:], 0.25)
                # esel[p, c, j] = (s4 == j) * ew  — folds sub-row select + weight
                for j in range(4):
                    nc.vector.tensor_scalar(
                        out=esel[:].rearrange("p (c j) -> p c j", j=4)[:, :, j],
                        in0=s4s[:], scalar1=float(j), scalar2=None,
                        op0=OP.is_equal)
                nc.vector.tensor_tensor(
                    out=esel[:].rearrange("p (c j) -> p c j", j=4),
                    in0=esel[:].rearrange("p (c j) -> p c j", j=4),
                    in1=ewf[:].unsqueeze(2).to_broadcast([P, SCOLS, 4]),
                    op=OP.mult)

                # ---- deg / dinv ----
                if "deg" in parts:
                    for t in range(NT):
                        lo, hi = int(coloff[t]), int(coloff[t + 1])
                        if hi == lo:
                            nc.vector.memset(deg[:, t:t + 1], 0)
                            continue
                        nc.vector.tensor_reduce(out=deg[:, t:t + 1],
                                                in_=ewf[:, lo:hi], axis=X,
                                                op=OP.add)
                    nc.vector.tensor_tensor(out=deg[:], in0=deg[:], in1=npcs[:],
                                            op=OP.subtract)
                else:
                    nc.vector.memset(deg[:], 3.0)
                nc.scalar.activation(out=scr[:], in_=deg[:], func=AF.Sqrt, bias=1.0)
                nc.vector.reciprocal(out=dinv[:], in_=scr[:])

                # ---- xs = dinv * (x @ Wc1) ----
                xT8 = erl.tile([P, NBP], FP8)
                nc.sync.dma_start(xT8[:], xT_d[:])
                xTt = erl.tile([P, NBP], BF16)
                nc.vector.tensor_copy(out=xTt[:], in_=xT8[:])
                for t0 in range(0, NT, 8):
                    nt8 = min(8, NT - t0)
                    ps = psX.tile([P, 512], F32, space="PSUM", tag="x")
                    for j in range(nt8):
                        nc.tensor.matmul(
                            out=ps[:, j * NFIL:(j + 1) * NFIL],
                            lhsT=xTt[:, (t0 + j) * P:(t0 + j + 1) * P],
                            rhs=Wc1s[:], start=True, stop=True)
                    nc.vector.tensor_copy(
                        out=xs[:, t0 * NFIL:(t0 + nt8) * NFIL],
                        in_=ps[:, :nt8 * NFIL])
                nc.vector.tensor_tensor(
                    out=xs[:].rearrange("p (t f) -> p t f", f=NFIL),
                    in0=xs[:].rearrange("p (t f) -> p t f", f=NFIL),
                    in1=dinv[:].unsqueeze(2).to_broadcast([P, NT, NFIL]),
                    op=OP.mult)

            nc.sync.dma_start(
                bounce1[:].rearrange("(t p) f -> p t f", p=P),
                xs[:].rearrange("p (t f) -> p t f", f=NFIL))
            if "ag1" in parts:
                nc.gpsimd.collective_compute(
                    "AllGather", OP.bypass, replica_groups=[list(range(NC))],
                    ins=[bounce1[:].opt()], outs=[table1[:].opt()])

            with tc.tile_pool(name="lay", bufs=2) as lay:
                # ---- gather + reduce layer ----
                def layer(table, accT):
                    for (wc0, wc1, tl) in windows:
                        w = wc1 - wc0
                        idxt = lay.tile([P, WMAX * 8], I16, tag="idx", bufs=4)
                        nc.sync.dma_start(idxt[:, :w * 8],
                                          idxw[:, wc0 * 8:wc1 * 8])
                        msgs = lay.tile([P, WMAX, 256], BF16, tag="mA")
                        nc.gpsimd.dma_gather(
                            out_ap=msgs[:, :w, :], in_ap=table[:],
                            idxs_ap=idxt[:, :w * 8], num_idxs=w * P,
                            num_idxs_reg=w * P, elem_size=256,
                            single_packet=False)
                        tmp = lay.tile([P, WMAX * 256], BF16, tag="tmp")
                        nc.vector.tensor_tensor(
                            out=tmp[:, :w * 256].rearrange(
                                "p (c j f) -> p c j f", j=4, f=NFIL),
                            in0=msgs[:, :w, :].rearrange(
                                "p c (j f) -> p c j f", f=NFIL),
                            in1=esel[:, wc0 * 4:wc1 * 4].rearrange(
                                "p (c j) -> p c j", j=4).unsqueeze(3)
                                .to_broadcast([P, w, 4, NFIL]),
                            op=OP.mult)
                        sc = lay.tile([P, WMAX, NFIL], BF16, tag="sc")
                        with nc.allow_low_precision(
                                reason="one-hot select: single nonzero term"):
                            nc.vector.tensor_reduce(
                                out=sc[:, :w, :],
                                in_=tmp[:, :w * 256].rearrange(
                                    "p (c j f) -> p c j f", j=4, f=NFIL)
                                    .transpose([0, 1, 3, 2]),
                                axis=X, op=OP.add)
                        for (t, lo, hi) in tl:
                            nc.vector.tensor_reduce(
                                out=accT[:, t * NFIL:(t + 1) * NFIL],
                                in_=sc[:, lo - wc0:hi - wc0, :].transpose(
                                    [0, 2, 1]),
                                axis=X, op=OP.add)
                    # tiles with zero columns never get written: zero them
                    for t in range(NT):
                        if coloff[t + 1] == coloff[t]:
                            nc.vector.memset(accT[:, t * NFIL:(t + 1) * NFIL], 0)

                if "l1" in parts:
                    layer(table1, acc)
                else:
                    nc.vector.memset(acc[:], 0)

                # ---- h1s = dinv*relu(dinv*(acc+xs)+bc1) ----
                nc.vector.tensor_tensor(out=acc[:], in0=acc[:], in1=xs[:],
                                        op=OP.add)
                nc.vector.tensor_tensor(
                    out=acc[:].rearrange("p (t f) -> p t f", f=NFIL),
                    in0=acc[:].rearrange("p (t f) -> p t f", f=NFIL),
                    in1=dinv[:].unsqueeze(2).to_broadcast([P, NT, NFIL]),
                    op=OP.mult)
                nc.vector.tensor_tensor(
                    out=acc[:].rearrange("p (t f) -> p t f", f=NFIL),
                    in0=acc[:].rearrange("p (t f) -> p t f", f=NFIL),
                    in1=bc1s[:].unsqueeze(1).to_broadcast([P, NT, NFIL]),
                    op=OP.add)
                nc.vector.tensor_scalar_max(acc[:], acc[:], 0.0)
                nc.vector.tensor_tensor(
                    out=h1s[:].rearrange("p (t f) -> p t f", f=NFIL),
                    in0=acc[:].rearrange("p (t f) -> p t f", f=NFIL),
                    in1=dinv[:].unsqueeze(2).to_broadcast([P, NT, NFIL]),
                    op=OP.mult)

                nc.sync.dma_start(
                    bounce2[:].rearrange("(t p) f -> p t f", p=P),
                    h1s[:].rearrange("p (t f) -> p t f", f=NFIL))
                if "ag2" in parts:
                    nc.gpsimd.collective_compute(
                        "AllGather", OP.bypass, replica_groups=[list(range(NC))],
                        ins=[bounce2[:].opt()], outs=[table2[:].opt()])

                nc.vector.memset(acc[:], 0)
                if "l2" in parts:
                    layer(table2, acc)

            # ---- final: log_softmax(dinv*((acc+h1s) @ Wc2) + bc2) ----
            nc.vector.tensor_tensor(out=acc[:], in0=acc[:], in1=h1s[:], op=OP.add)
            with tc.tile_pool(name="fin", bufs=1) as fin:
                for t0 in range(0, NT, 8):
                    nt8 = min(8, NT - t0)
                    tmp = fin.tile([P, 8 * CLS * NFIL], F32, tag="tmp")
                    nc.vector.tensor_tensor(
                        out=tmp[:, :nt8 * CLS * NFIL].rearrange(
                            "p (t k f) -> p t k f", k=CLS, f=NFIL),
                        in0=acc[:, t0 * NFIL:(t0 + nt8) * NFIL].rearrange(
                            "p (t f) -> p t f", f=NFIL).unsqueeze(2)
                            .to_broadcast([P, nt8, CLS, NFIL]),
                        in1=wc2fs[:].rearrange("p (k f) -> p k f", f=NFIL)
                            .unsqueeze(1).to_broadcast([P, nt8, CLS, NFIL]),
                        op=OP.mult)
                    nc.vector.tensor_reduce(
                        out=zpre[:, t0 * CLS:(t0 + nt8) * CLS],
                        in_=tmp[:, :nt8 * CLS * NFIL].rearrange(
                            "p (c f) -> p c f", f=NFIL),
                        axis=X, op=OP.add)
                nc.vector.tensor_tensor(
                    out=zpre[:].rearrange("p (t k) -> p t k", k=CLS),
                    in0=zpre[:].rearrange("p (t k) -> p t k", k=CLS),
                    in1=dinv[:].unsqueeze(2).to_broadcast([P, NT, CLS]),
                    op=OP.mult)
                nc.vector.tensor_tensor(
                    out=zpre[:].rearrange("p (t k) -> p t k", k=CLS),
                    in0=zpre[:].rearrange("p (t k) -> p t k", k=CLS),
                    in1=bc2s[:].unsqueeze(1).to_broadcast([P, NT, CLS]),
                    op=OP.add)
                nmx = fin.tile([P, NT], F32)
                nc.vector.tensor_reduce(
                    out=nmx[:], in_=zpre[:].rearrange("p (t k) -> p t k", k=CLS),
                    axis=X, op=OP.max, negate=True)
                nc.vector.tensor_tensor(
                    out=zpre[:].rearrange("p (t k) -> p t k", k=CLS),
                    in0=zpre[:].rearrange("p (t k) -> p t k", k=CLS),
                    in1=nmx[:].unsqueeze(2).to_broadcast([P, NT, CLS]), op=OP.add)
                et = fin.tile([P, NT * CLS], F32)
                nc.scalar.activation(out=et[:], in_=zpre[:], func=AF.Exp)
                sume = fin.tile([P, NT], F32)
                nc.vector.tensor_reduce(
                    out=sume[:], in_=et[:].rearrange("p (t k) -> p t k", k=CLS),
                    axis=X, op=OP.add)
                lse = fin.tile([P, NT], F32)
                nc.scalar.activation(out=lse[:], in_=sume[:], func=AF.Ln)
                res = fin.tile([P, NT * CLS], F16)
                nc.vector.tensor_tensor(
                    out=res[:].rearrange("p (t k) -> p t k", k=CLS),
                    in0=zpre[:].rearrange("p (t k) -> p t k", k=CLS),
                    in1=lse[:].unsqueeze(2).to_broadcast([P, NT, CLS]),
                    op=OP.subtract)
                nc.sync.dma_start(
                    out_d[:].rearrange("(t p) f -> p t f", p=P),
                    res[:].rearrange("p (t f) -> p t f", f=CLS))

    nc.compile()
    return nc


_last = {}


def kernel(**inputs):
    in_maps, consts, meta = _prep(**inputs)
    nc = _build(consts, meta)
    _last.update(nc=nc, in_maps=in_maps, meta=meta)
    res = run_bass_kernel_spmd(nc, in_maps, core_ids=list(range(NC)))
    _last["exec_time_ns"] = getattr(res, "exec_time_ns", None)
    out = np.zeros((N, CLS), np.float32)
    for k in range(NC):
        ok = res.results[k]["out"].astype(np.float32)   # [NBP, CLS] sorted order
        perm = meta["perms"][k]
        valid = perm < NB
        out[k * NB + perm[valid]] = ok[valid.nonzero()[0]]
    return out
